# revision 100
# baseline (speedup 1.0000x reference)
"""Disentangled self-attention (DeBERTa-style) on 8 TRN2 NeuronCores.

Problem: B=4, L=256, D=512, H=8, R=64 rel-pos buckets, DK=64.
Sharding: core c handles batch b=c//2, query rows l0=128*(c%2) .. l0+128.
No cross-core communication (output rows are disjoint).

The kernel is HBM-wire-bound (~24MB/core), so everything is organized
around keeping the DMA queues streaming:
  - rel_v streams as fp8(e4m3) in 8 grouped DMAs (16 query rows each,
    host-transposed so every partition reads one contiguous 16KB run),
    split across the SP and Pool issue queues
  - the gather one-hots O1/O2 are fp8 and *live in the rel_v tile pool*:
    after t1/t2 consume them, the pool rotation hands their buffers to
    rel_v groups 6/7, so all 8 groups stream with no softmax dependency
  - constants are packed into 4 DMAs (Wall/xall/brow/rels); q/k biases
    are applied as per-partition scalar adds on the PSUM->SBUF copies
    instead of rank-1 matmuls; v bias via a replicated-row tensor add
  - scores psum A[l,h,m] = rank-1 key-mask seed + q.k per head +
    PE-transposed content->position term (t1); position->content (t2)
    goes to psum B[l,m,h], staged to SBUF in m-quarters (HW: DVE cannot
    read two PSUM operands; also matmuls in one accumulation group must
    not alternate operand base partitions)
  - c2p/p2c tables are written to PSUM partitions (j*64+r) via base-64
    matmul outputs, so the block-diagonal pair rhs assembles with plain
    same-partition copies
  - softmax skips the max-subtraction (scores bounded ~|2.5|); exp on
    ACT with fused sum, normalize on DVE; copies are load-balanced
    across ACT and DVE throughout
  - ctx via head-pair psums [128, 2*128]: v-seed matmul + one
    [128x128]@[128x2] fp8 matmul per (query row, head pair, key chunk)
  - output projection in column halves so DMA overlaps the second half
"""

import sys

for _p in ("/opt/trn_rl_repo", "/root/.axon_site/_ro/trn_rl_repo"):
    if _p not in sys.path:
        sys.path.append(_p)

import numpy as np

import concourse.bass as bass
import concourse.tile as tile
from concourse import bacc, mybir
from concourse.bass_utils import run_bass_kernel_spmd
from concourse.masks import make_identity

B, L, D, H = 4, 256, 512, 8
R = 64
DK = D // H
LH = 128                      # query rows per core
NCORES = 8
SCALE = float(1.0 / (3.0 * np.sqrt(np.float32(DK))))
MASKVAL = -60000.0            # exp() underflows identically to the ref's -1e9

F16 = mybir.dt.float16
F32 = mybir.dt.float32
F8 = mybir.dt.float8e4
EXP = mybir.ActivationFunctionType.Exp
COPY = mybir.ActivationFunctionType.Copy
AX = mybir.AxisListType.X

RVG = 16                      # query rows per grouped rel_v DMA
RV_SP = (0, 1, 2, 3, 6)       # rel_v groups issued from SP (rest from Pool)


def build_nc(phase=99, sub="all"):
    nc = bacc.Bacc(None, target_bir_lowering=False)

    # ---- DRAM I/O (per-core shard shapes) ----
    # packed constants: one DMA per tensor (issue-time dominated otherwise)
    # Wall[:, 0:4]=WqT chunks, 4:8=WkT, 8:12=WvT, 12:16=WoT
    d_Wall = nc.dram_tensor("Wall", [128, 16, D], F16, kind="ExternalInput")
    # xall cols: qT chunks 0:512, kT 512:1536, vT 1536:2560
    d_xall = nc.dram_tensor("xall", [128, 2560], F16, kind="ExternalInput")
    # brow cols: mask-pair 0:512, ones 512:768 (single-partition rows are
    # expensive to DMA -- keep this tile small)
    d_brow = nc.dram_tensor("brow", [1, 768], F16, kind="ExternalInput")
    # per-partition bias tables: bqk[d, 0:8]=bq*S by head, [d, 8:16]=bk
    d_bqk = nc.dram_tensor("bqk", [DK, 16], F32, kind="ExternalInput")
    d_bvr = nc.dram_tensor("bvr", [128, D], F16, kind="ExternalInput")
    d_rels = nc.dram_tensor("rels", [DK, 2, H, R], F16, kind="ExternalInput")
    # stacked-pair one-hots: row r + 64*j holds pair-member j (l=2p+j / m=2p+j)
    # stacked-pair one-hots: partition r + 64*j holds pair-member j
    d_O1 = nc.dram_tensor("O1", [128, LH // 2, L], F8, kind="ExternalInput")
    d_O2 = nc.dram_tensor("O2", [128, L // 2, LH], F8, kind="ExternalInput")
    # rel_v pre-transposed on host: (gq, p, g, c, f) = rv[gq*RVG+g, c*128+p, f]
    d_rv = nc.dram_tensor("rv", [LH // RVG, 128, RVG, 2, D], F8,
                          kind="ExternalInput")
    d_out = nc.dram_tensor("out", [LH, D], F32, kind="ExternalOutput")

    with tile.TileContext(nc) as tc:
        with (
            tc.tile_pool(name="consts", bufs=1) as consts,
            tc.tile_pool(name="work", bufs=1) as work,
            tc.tile_pool(name="sm", bufs=2) as smp,
            tc.tile_pool(name="rvp", bufs=8) as rvp,
        ):
            dbg_ap = None

            # ---------- constants into SBUF ----------
            def load(name, dram, shape, dtype=F16):
                t = consts.tile(shape, dtype, tag=name, name=name)
                nc.gpsimd.dma_start(out=t[:], in_=dram)
                return t

            xall = load("xall", d_xall[:, :], [128, 2560])
            wall = consts.tile([128, 16, D], F16, tag="wall", name="wall")
            nc.gpsimd.dma_start(out=wall[:, 0:8, :], in_=d_Wall[:, 0:8, :])
            bqk = load("bqk", d_bqk[:, :], [DK, 16], F32)
            rels = load("rels", d_rels[:, :, :, :], [DK, 2, H, R])
            brow = load("brow", d_brow[:, :], [1, 768])
            nc.gpsimd.dma_start(out=wall[:, 8:16, :], in_=d_Wall[:, 8:16, :])
            bvr = load("bvr", d_bvr[:, :], [128, D])

            wq = [wall[:, i, :] for i in range(4)]
            wk = [wall[:, 4 + i, :] for i in range(4)]
            wv = [wall[:, 8 + i, :] for i in range(4)]
            wo = [wall[:, 12 + i, :] for i in range(4)]
            mask2 = brow[:, 0:512]
            ones16 = brow[:, 512:512 + LH]
            xq = [xall[:, i * LH:(i + 1) * LH] for i in range(4)]
            xk = [xall[:, 512 + i * L:512 + (i + 1) * L] for i in range(4)]
            xv = [xall[:, 1536 + i * L:1536 + (i + 1) * L] for i in range(4)]
            rkT = rels[:, 0]
            rqT = rels[:, 1]

            # one-hots go on the Activation issue queue so the big transfers
            # don't block the Pool const stream. They live in the rvp pool
            # (same 16KB as an rv group tile): once t1/t2 consume them, the
            # pool rotation hands their buffers to rel_v groups 6 and 7 --
            # all 8 groups stream without waiting for softmax/pT.
            o1t = rvp.tile([128, LH // 2, L], F8, tag="rv", name="o1t")
            nc.scalar.dma_start(out=o1t[:], in_=d_O1[:, :, :])
            o2t = rvp.tile([128, L // 2, LH], F8, tag="rv", name="o2t")
            nc.scalar.dma_start(out=o2t[:], in_=d_O2[:, :, :])

            id16 = consts.tile([128, 128], F16, tag="id16")
            make_identity(nc, id16[:])
            id32 = consts.tile([128, 128], F32, tag="id32")
            make_identity(nc, id32[:])

            if phase == 0:
                dbg_ap = xq[0][:]

            # ---------- projections ----------
            if phase >= 1:
                qf2 = work.tile([DK, H, LH], F16, tag="qf2", name="qf2")
                kf2 = work.tile([DK, H, L], F16, tag="kf2", name="kf2")
                vp = [work.tile([128, D], F16, tag=f"vp{i}", name=f"vp{i}") for i in range(2)]

                with tc.tile_pool(name="pproj", bufs=4, space="PSUM") as pproj:
                    for h in range(H):
                        hs = slice(h * DK, (h + 1) * DK)
                        ps = pproj.tile([DK, LH], F32, tag="pp", name="pp")
                        for kc in range(4):
                            nc.tensor.matmul(ps[:], wq[kc][:, hs],
                                             xq[kc][:], start=(kc == 0),
                                             stop=(kc == 3))
                        nc.vector.tensor_scalar_add(qf2[:, h, :], ps[:],
                                                    bqk[:, h:h + 1])
                    for h in range(H):
                        hs = slice(h * DK, (h + 1) * DK)
                        ps = pproj.tile([DK, L], F32, tag="pp", name="pp")
                        for kc in range(4):
                            nc.tensor.matmul(ps[:], wk[kc][:, hs],
                                             xk[kc][:], start=(kc == 0),
                                             stop=(kc == 3))
                        nc.vector.tensor_scalar_add(kf2[:, h, :], ps[:],
                                                    bqk[:, 8 + h:9 + h])
                    # c2p/p2c: psum halves land on partitions (j*64+r) via
                    # base-64 matmul writes, so the block-diagonal pair rhs
                    # assembles with plain same-partition copies (no shift DMA)
                    c2p2 = work.tile([128, 16, LH // 2], F16, tag="c2p2", name="c2p2")
                    p2c2 = work.tile([128, 16, L // 2], F16, tag="p2c2", name="p2c2")
                    nc.vector.memset(c2p2[:], 0.0)
                    nc.vector.memset(p2c2[:], 0.0)
                    with tc.tile_pool(name="ppc", bufs=1, space="PSUM") as ppc:
                        pc2 = ppc.tile([128, H, LH // 2], F32, tag="pc2", name="pc2")
                        pp2 = ppc.tile([128, H, L // 2], F32, tag="pp2", name="pp2")
                        for h in range(H):
                            qj = qf2[:, h, :].rearrange("d (p j) -> d j p", j=2)
                            kj = kf2[:, h, :].rearrange("d (p j) -> d j p", j=2)
                            for j in range(2):
                                nc.tensor.matmul(pc2[64 * j:64 * j + 64, h, :],
                                                 rkT[:, h, :], qj[:, j, :],
                                                 start=True, stop=True)
                                nc.tensor.matmul(pp2[64 * j:64 * j + 64, h, :],
                                                 rqT[:, h, :], kj[:, j, :],
                                                 start=True, stop=True)
                        # split the 4 assembly copies across ACT and DVE
                        nc.scalar.activation(c2p2[0:64, 0:8, :], pc2[0:64, :, :], COPY)
                        nc.scalar.activation(c2p2[64:128, 8:16, :], pc2[64:128, :, :],
                                             COPY)
                        nc.vector.tensor_copy(p2c2[0:64, 0:8, :], pp2[0:64, :, :])
                        nc.vector.tensor_copy(p2c2[64:128, 8:16, :], pp2[64:128, :, :])

                    for mc in range(2):
                        ps = pproj.tile([128, D], F32, tag="pp", name="pp")
                        for kc in range(4):
                            nc.tensor.matmul(ps[:], xv[kc][:, mc * 128:(mc + 1) * 128],
                                             wv[kc][:], start=(kc == 0),
                                             stop=(kc == 3))
                        nc.vector.tensor_add(vp[mc][:], ps[:], bvr[:])

                if phase == 1:
                    dbg_ap = qf2[:, 0, :]

            # ---------- scores + softmax ----------
            _lv = {"qk": 0, "t1": 1, "tr": 2, "B": 3, "sm": 4, "all": 9}[sub]
            if phase >= 2:
                with tc.tile_pool(name="pscore", bufs=1, space="PSUM") as pscore:
                    A = pscore.tile([128, H, L], F32, tag="A", name="A")    # 4 banks

                    # t1 first: psum t1T[mc][m, l, h], both chunks in one big
                    # tile (no WAR between chunks) -> ACT-copy to sbuf ->
                    # PE-transpose into A
                    t1s = [work.tile([128, LH, H], F32, tag=f"t1s{mc}", name=f"t1s{mc}")
                           for mc in range(2)]
                    t1 = pscore.tile([128, 2, LH, H], F32, tag="big", name="big")
                    for mc in range(2 if _lv >= 1 else 0):
                        for p in range(LH // 2):
                            nc.tensor.matmul(t1[:, mc, 2 * p:2 * p + 2, :],
                                             o1t[:, p, mc * 128:(mc + 1) * 128],
                                             c2p2[:, :, p],
                                             start=(p % 32 == 0), stop=(p % 32 == 31))
                        nc.scalar.activation(t1s[mc][:], t1[:, mc], COPY)

                    # mask seeds each bank (start=True covers 2 heads), then
                    # qk accumulates -- overlaps the ACT t1s copies
                    for h2 in range(0, H, 2):
                        nc.tensor.matmul(A[:, h2:h2 + 2, :], ones16,
                                         mask2, start=True, stop=False)
                    for h in range(H):
                        nc.tensor.matmul(A[:, h, :], qf2[:, h, :], kf2[:, h, :],
                                         start=False,
                                         stop=(_lv < 2 and h % 2 == 1))

                    # term2 before the A-transposes: B[l, m, h] one-hot
                    # matmuls, staged to SBUF in m-quarters as each accum
                    # group closes (DVE cannot read two PSUM operands on HW)
                    Bp = pscore.tile([128, L, H], F32, tag="big", name="big")
                    B_sb = work.tile([128, L, H], F16, tag="B_sb", name="B_sb")
                    for p in range(L // 2 if _lv >= 3 else 0):
                        nc.tensor.matmul(Bp[:, 2 * p:2 * p + 2, :], o2t[:, p, :],
                                         p2c2[:, :, p],
                                         start=(p % 32 == 0), stop=(p % 32 == 31))
                        if p % 32 == 31:
                            q0 = (p - 31) * 2
                            nc.vector.tensor_copy(B_sb[:, q0:q0 + 64, :],
                                                  Bp[:, q0:q0 + 64, :])

                    for mc in range(2 if _lv >= 2 else 0):
                        for h in range(H):
                            nc.tensor.matmul(A[:, h, mc * 128:(mc + 1) * 128],
                                             t1s[mc][:, :, h], id32[:],
                                             is_transpose=True, start=False,
                                             stop=(mc == 1 and h % 2 == 1))
                    # softmax per head (A from PSUM, B from SBUF).
                    # scores are bounded (~|5|) so exp needs no max-subtract;
                    # normalization cancels the missing shift exactly.
                    p16 = work.tile([128, H, L], F16, tag="p16", name="p16")
                    sums = work.tile([128, H], F32, tag="sums", name="sums")
                    recs = work.tile([128, H], F32, tag="recs", name="recs")
                    for h in range(H if _lv >= 4 else 0):
                        s = smp.tile([128, L], F16, tag="s", name="s")
                        nc.vector.tensor_add(s[:], A[:, h, :], B_sb[:, :, h])
                        e = smp.tile([128, L], F16, tag="e", name="e")
                        nc.scalar.activation(e[:], s[:], EXP, scale=1.0,
                                             accum_out=sums[:, h:h + 1])
                        nc.vector.reciprocal(recs[:, h:h + 1], sums[:, h:h + 1])
                        nc.vector.tensor_scalar_mul(p16[:, h, :], e[:], recs[:, h:h + 1])

                if phase == 2:
                    dbg_ap = {0: A[:, 0, :], 1: t1s[0][:, :, 0], 2: A[:, 0, :],
                              3: p16[:, 0, :], 4: p16[:, 0, :], 9: p16[:, 0, :]}[_lv]
                if phase == 3:
                    dbg_ap = p16[:, 0, :]

            # ---------- ctx + output projection ----------
            if phase >= 4:
                with (
                    tc.tile_pool(name="pctx", bufs=1, space="PSUM") as pctx,
                    tc.tile_pool(name="ppt", bufs=2, space="PSUM") as ppt,
                ):
                    pT = [work.tile([128, H, LH], F16, tag=f"pT{c}", name=f"pT{c}")
                          for c in range(2)]
                    for c in range(2):
                        for h in range(H):
                            pps = ppt.tile([128, 128], F16, tag="pt", name="pt")
                            nc.tensor.matmul(pps[:], p16[:, h, c * 128:(c + 1) * 128],
                                             id16[:], is_transpose=True)
                            if c == 1 and h < 6:
                                nc.scalar.activation(pT[c][:, h, :], pps[:], COPY)
                            else:
                                nc.vector.tensor_copy(pT[c][:, h, :], pps[:])

                    cp = [pctx.tile([128, 2 * LH], F32, tag=f"cp{hp}", name=f"cp{hp}")
                          for hp in range(4)]
                    nrv = LH // RVG if phase >= 5 else 0
                    for hp in range(4):
                        for c in range(2):
                            rhs = pT[c][:, 2 * hp:2 * hp + 2, :].rearrange("p hh l -> p l hh")
                            nc.tensor.matmul(cp[hp][:], vp[c][:, hp * 128:(hp + 1) * 128],
                                             rhs, start=(c == 0),
                                             stop=(c == 1 and nrv == 0))
                    for gq in range(nrv):
                        rvt = rvp.tile([128, RVG, 2, D], F8, tag="rv", name="rv")
                        eng = nc.sync if gq in RV_SP else nc.gpsimd
                        eng.dma_start(out=rvt[:], in_=d_rv[gq])
                        for g in range(RVG):
                            l = gq * RVG + g
                            for hp in range(4):
                                for c in range(2):
                                    nc.tensor.matmul(
                                        cp[hp][:, 2 * l:2 * l + 2],
                                        rvt[:, g, c, hp * 128:(hp + 1) * 128],
                                        pT[c][:, 2 * hp:2 * hp + 2, l:l + 1],
                                        start=False, stop=(c == 1 and l == LH - 1))

                    ctxT = [work.tile([128, LH], F16, tag=f"ctxT{hp}", name=f"ctxT{hp}")
                            for hp in range(4)]
                    for hp in range(4):
                        eng = nc.vector.tensor_copy if hp < 2 else (
                            lambda o, i: nc.scalar.activation(o, i, COPY))
                        eng(ctxT[hp][0:64, :],
                            cp[hp][0:64, :].rearrange("p (l hh) -> p hh l", hh=2)[:, 0, :])
                        eng(ctxT[hp][64:128, :],
                            cp[hp][64:128, :].rearrange("p (l hh) -> p hh l", hh=2)[:, 1, :])
                    # output projection in column halves so the first half's
                    # DMA overlaps the second half's matmuls
                    out_sb = work.tile([128, D], F32, tag="out_sb", name="out_sb")
                    for half in range(2):
                        cs = slice(half * 256, half * 256 + 256)
                        ops = pctx.tile([128, 256], F32, tag=f"oh{half}",
                                        name=f"oh{half}")
                        for hp in range(4):
                            nc.tensor.matmul(ops[:], ctxT[hp][:], wo[hp][:, cs],
                                             start=(hp == 0), stop=(hp == 3))
                        if half == 0:
                            nc.vector.tensor_copy(out_sb[:, cs], ops[:])
                        else:
                            nc.scalar.activation(out_sb[:, cs], ops[:], COPY)
                        nc.sync.dma_start(out=d_out[:, cs], in_=out_sb[:, cs])

            if phase < 4:
                dbg = work.tile([128, D], F32, tag="dbg", name="dbg")
                nc.vector.memset(dbg[:], 0.0)
                n = min(int(np.prod(dbg_ap.shape[1:])), D)
                nc.vector.tensor_copy(dbg[:dbg_ap.shape[0], 0:n], dbg_ap[:, 0:n])
                nc.sync.dma_start(out=d_out[:, :], in_=dbg[:])

    nc.finalize()
    return nc


_NC_CACHE = None


def _get_nc():
    global _NC_CACHE
    if _NC_CACHE is None:
        import os
        _NC_CACHE = build_nc(int(os.environ.get("KPHASE", "99")),
                             os.environ.get("KSUB", "all"))
    return _NC_CACHE


def host_prep(inputs, c):
    import ml_dtypes
    f8 = ml_dtypes.float8_e4m3
    f16, f32 = np.float16, np.float32
    b, lh = c // 2, c % 2
    l0 = lh * LH
    q = np.asarray(inputs["query"][b], f32)
    k = np.asarray(inputs["key"][b], f32)
    v = np.asarray(inputs["value"][b], f32)
    mask = np.asarray(inputs["mask"][b])
    rp = np.asarray(inputs["rel_pos"][b], np.int64)
    rv = np.asarray(inputs["rel_v"][b], f32)

    d = {}
    qT = (q[l0:l0 + LH].T * SCALE).astype(f16)          # [512, LH]
    kT = k.T.astype(f16)                                # [512, L]
    vT = v.T.astype(f16)                                # [512, L]
    d["xall"] = np.ascontiguousarray(np.concatenate(
        [qT.reshape(4, 128, LH).transpose(1, 0, 2).reshape(128, 512),
         kT.reshape(4, 128, L).transpose(1, 0, 2).reshape(128, 1024),
         vT.reshape(4, 128, L).transpose(1, 0, 2).reshape(128, 1024)], axis=1))
    WqT = np.asarray(inputs["Wq"], f32).T.astype(f16)
    WkT = np.asarray(inputs["Wk"], f32).T.astype(f16)
    WvT = np.asarray(inputs["Wv"], f32).T.astype(f16)
    WoT = np.asarray(inputs["Wo"], f32).T.astype(f16)
    d["Wall"] = np.ascontiguousarray(np.concatenate(
        [W.reshape(4, 128, D) for W in (WqT, WkT, WvT, WoT)],
        axis=0).transpose(1, 0, 2))
    mrow = np.where(mask, np.float16(MASKVAL), np.float16(0.0)).astype(f16)
    d["brow"] = np.concatenate([np.tile(mrow, 2), np.ones((256,), f16)])[None, :]
    d["bqk"] = np.ascontiguousarray(np.concatenate(
        [(np.asarray(inputs["bq"], f32) * SCALE).reshape(H, DK).T,
         np.asarray(inputs["bk"], f32).reshape(H, DK).T], axis=1)).astype(f32)
    d["bvr"] = np.tile(np.asarray(inputs["bv"], f32).astype(f16), (128, 1))
    d["rels"] = np.ascontiguousarray(np.stack(
        [np.asarray(inputs["rel_k"], f32).transpose(2, 0, 1),
         np.asarray(inputs["rel_q"], f32).transpose(2, 0, 1) * SCALE],
        axis=1)).astype(f16)
    rp_c = rp[l0:l0 + LH]
    eye = np.eye(R, dtype=f16)
    O1 = eye[:, rp_c]                                   # [R, LH, L]
    O2 = eye[:, rp_c.T]                                 # [R, L, LH]
    d["O1"] = np.ascontiguousarray(
        O1.reshape(R, LH // 2, 2, L).transpose(2, 0, 1, 3).reshape(128, LH // 2, L)
    ).astype(f8)
    d["O2"] = np.ascontiguousarray(
        O2.reshape(R, L // 2, 2, LH).transpose(2, 0, 1, 3).reshape(128, L // 2, LH)
    ).astype(f8)
    # (gq, p, g, c, f) = rv[gq*RVG+g, c*128+p, f] so each partition's group
    # payload is one contiguous run (big DMA descriptors)
    rv8 = rv[l0:l0 + LH].astype(f8)                     # [LH, L, D]
    d["rv"] = np.ascontiguousarray(
        rv8.reshape(LH // RVG, RVG, 2, 128, D).transpose(0, 3, 1, 2, 4))
    return d


def kernel(**inputs) -> np.ndarray:
    nc = _get_nc()
    in_maps = [host_prep(inputs, c) for c in range(NCORES)]
    res = run_bass_kernel_spmd(nc, in_maps, core_ids=list(range(NCORES)))
    out = np.zeros((B, L, D), np.float32)
    for c in range(NCORES):
        b, lh = c // 2, c % 2
        out[b, lh * LH:(lh + 1) * LH] = res.results[c]["out"]
    out += np.asarray(inputs["bo"], np.float32)[None, None, :]
    return out



# revision 107
# speedup vs baseline: 1.0903x; 1.0903x over previous
"""Disentangled self-attention (DeBERTa-style) on 8 TRN2 NeuronCores.

Problem: B=4, L=256, D=512, H=8, R=64 rel-pos buckets, DK=64.
Sharding: core c handles batch b=c//2, query rows l0=128*(c%2) .. l0+128.
No cross-core communication (output rows are disjoint).

The kernel is HBM-wire-bound (~24MB/core), so everything is organized
around keeping the DMA queues streaming:
  - rel_v streams as fp8(e4m3) in 8 grouped DMAs (16 query rows each,
    host-transposed so every partition reads one contiguous 16KB run),
    split across the SP and Pool issue queues
  - the gather one-hots O1/O2 are fp8 and *live in the rel_v tile pool*:
    after t1/t2 consume them, the pool rotation hands their buffers to
    rel_v groups 6/7, so all 8 groups stream with no softmax dependency
  - constants are packed into 4 DMAs (Wall/xall/brow/rels); q/k biases
    are applied as per-partition scalar adds on the PSUM->SBUF copies
    instead of rank-1 matmuls; v bias via a replicated-row tensor add
  - scores psum A[l,h,m] = rank-1 key-mask seed + q.k per head +
    PE-transposed content->position term (t1); position->content (t2)
    goes to psum B[l,m,h], staged to SBUF in m-quarters (HW: DVE cannot
    read two PSUM operands; also matmuls in one accumulation group must
    not alternate operand base partitions)
  - c2p/p2c tables are written to PSUM partitions (j*64+r) via base-64
    matmul outputs, so the block-diagonal pair rhs assembles with plain
    same-partition copies
  - softmax skips the max-subtraction (scores bounded ~|2.5|); exp on
    ACT with fused sum, normalize on DVE; copies are load-balanced
    across ACT and DVE throughout
  - ctx via head-pair psums [128, 2*128]: v-seed matmul + one
    [128x128]@[128x2] fp8 matmul per (query row, head pair, key chunk)
  - output projection in column halves so DMA overlaps the second half
"""

import sys

for _p in ("/opt/trn_rl_repo", "/root/.axon_site/_ro/trn_rl_repo"):
    if _p not in sys.path:
        sys.path.append(_p)

import numpy as np

import concourse.bass as bass
import concourse.tile as tile
from concourse import bacc, mybir
from concourse.bass_utils import run_bass_kernel_spmd
from concourse.masks import make_identity

B, L, D, H = 4, 256, 512, 8
R = 64
DK = D // H
LH = 128                      # query rows per core
NCORES = 8
SCALE = float(1.0 / (3.0 * np.sqrt(np.float32(DK))))
MASKVAL = -60000.0            # exp() underflows identically to the ref's -1e9

F16 = mybir.dt.float16
F32 = mybir.dt.float32
F8 = mybir.dt.float8e4
EXP = mybir.ActivationFunctionType.Exp
COPY = mybir.ActivationFunctionType.Copy
AX = mybir.AxisListType.X

RVG = 16                      # query rows per grouped rel_v DMA
RV_SP = (0, 1, 2, 3, 6)       # rel_v groups issued from SP (rest from Pool)


def build_nc(phase=99, sub="all"):
    nc = bacc.Bacc(None, target_bir_lowering=False)

    # ---- DRAM I/O (per-core shard shapes) ----
    # packed constants: one DMA per tensor (issue-time dominated otherwise)
    # Wall[:, 0:4]=WqT chunks, 4:8=WkT, 8:12=WvT, 12:16=WoT
    d_Wall = nc.dram_tensor("Wall", [128, 16, D], F16, kind="ExternalInput")
    # xall cols: qT chunks 0:512, kT 512:1536, vT 1536:2560
    d_xall = nc.dram_tensor("xall", [128, 2560], F16, kind="ExternalInput")
    # brow cols: mask-pair 0:512, ones 512:768 (single-partition rows are
    # expensive to DMA -- keep this tile small)
    d_brow = nc.dram_tensor("brow", [1, 768], F16, kind="ExternalInput")
    # per-partition bias tables: bqk[d, 0:8]=bq*S by head, [d, 8:16]=bk
    d_bqk = nc.dram_tensor("bqk", [DK, 16], F32, kind="ExternalInput")
    d_bvr = nc.dram_tensor("bvr", [128, D], F16, kind="ExternalInput")
    d_rels = nc.dram_tensor("rels", [DK, 2, H, R], F16, kind="ExternalInput")
    # stacked-pair one-hots: row r + 64*j holds pair-member j (l=2p+j / m=2p+j)
    # stacked-pair one-hots: partition r + 64*j holds pair-member j
    d_O1 = nc.dram_tensor("O1", [128, LH // 2, L], F8, kind="ExternalInput")
    d_O2 = nc.dram_tensor("O2", [128, L // 2, LH], F8, kind="ExternalInput")
    # rel_v pre-transposed on host: (gq, p, g, c, f) = rv[gq*RVG+g, c*128+p, f]
    d_rv = nc.dram_tensor("rv", [LH // RVG, 128, RVG, 2, D], F8,
                          kind="ExternalInput")
    d_out = nc.dram_tensor("out", [LH, D], F32, kind="ExternalOutput")

    with tile.TileContext(nc) as tc:
        with (
            tc.tile_pool(name="consts", bufs=1) as consts,
            tc.tile_pool(name="work", bufs=1) as work,
            tc.tile_pool(name="sm", bufs=2) as smp,
            tc.tile_pool(name="rvp", bufs=8) as rvp,
        ):
            dbg_ap = None

            # ---------- constants into SBUF ----------
            def load(name, dram, shape, dtype=F16):
                t = consts.tile(shape, dtype, tag=name, name=name)
                nc.gpsimd.dma_start(out=t[:], in_=dram)
                return t

            xall = consts.tile([128, 2560], F16, tag="xall", name="xall")
            wall = consts.tile([128, 16, D], F16, tag="wall", name="wall")
            # q pieces first, then k, then v/o -- each projection starts as
            # soon as its own operands land
            nc.gpsimd.dma_start(out=xall[:, 0:512], in_=d_xall[:, 0:512])
            nc.gpsimd.dma_start(out=wall[:, 0:4, :], in_=d_Wall[:, 0:4, :])
            bqk = load("bqk", d_bqk[:, :], [DK, 16], F32)
            nc.gpsimd.dma_start(out=xall[:, 512:1536], in_=d_xall[:, 512:1536])
            nc.gpsimd.dma_start(out=wall[:, 4:8, :], in_=d_Wall[:, 4:8, :])
            rels = load("rels", d_rels[:, :, :, :], [DK, 2, H, R])
            brow = load("brow", d_brow[:, :], [1, 768])
            nc.gpsimd.dma_start(out=xall[:, 1536:2560], in_=d_xall[:, 1536:2560])
            nc.gpsimd.dma_start(out=wall[:, 8:16, :], in_=d_Wall[:, 8:16, :])
            bvr = load("bvr", d_bvr[:, :], [128, D])

            wq = [wall[:, i, :] for i in range(4)]
            wk = [wall[:, 4 + i, :] for i in range(4)]
            wv = [wall[:, 8 + i, :] for i in range(4)]
            wo = [wall[:, 12 + i, :] for i in range(4)]
            mask2 = brow[:, 0:512]
            ones16 = brow[:, 512:512 + LH]
            xq = [xall[:, i * LH:(i + 1) * LH] for i in range(4)]
            xk = [xall[:, 512 + i * L:512 + (i + 1) * L] for i in range(4)]
            xv = [xall[:, 1536 + i * L:1536 + (i + 1) * L] for i in range(4)]
            rkT = rels[:, 0]
            rqT = rels[:, 1]

            # one-hots go on the Activation issue queue so the big transfers
            # don't block the Pool const stream. They live in the rvp pool
            # (same 16KB as an rv group tile): once t1/t2 consume them, the
            # pool rotation hands their buffers to rel_v groups 6 and 7 --
            # all 8 groups stream without waiting for softmax/pT.
            o1t = rvp.tile([128, LH // 2, L], F8, tag="rv", name="o1t")
            nc.scalar.dma_start(out=o1t[:], in_=d_O1[:, :, :])
            o2t = rvp.tile([128, L // 2, LH], F8, tag="rv", name="o2t")
            nc.scalar.dma_start(out=o2t[:], in_=d_O2[:, :, :])

            id16 = consts.tile([128, 128], F16, tag="id16")
            make_identity(nc, id16[:])
            id32 = consts.tile([128, 128], F32, tag="id32")
            make_identity(nc, id32[:])

            if phase == 0:
                dbg_ap = xq[0][:]

            # ---------- projections ----------
            if phase >= 1:
                qf2 = work.tile([DK, H, LH], F16, tag="qf2", name="qf2")
                kf2 = work.tile([DK, H, L], F16, tag="kf2", name="kf2")
                vp = [work.tile([128, D], F16, tag=f"vp{i}", name=f"vp{i}") for i in range(2)]

                with tc.tile_pool(name="pproj", bufs=4, space="PSUM") as pproj:
                    for h in range(H):
                        hs = slice(h * DK, (h + 1) * DK)
                        ps = pproj.tile([DK, LH], F32, tag="pp", name="pp")
                        for kc in range(4):
                            nc.tensor.matmul(ps[:], wq[kc][:, hs],
                                             xq[kc][:], start=(kc == 0),
                                             stop=(kc == 3))
                        nc.vector.tensor_scalar_add(qf2[:, h, :], ps[:],
                                                    bqk[:, h:h + 1])
                    for h in range(H):
                        hs = slice(h * DK, (h + 1) * DK)
                        ps = pproj.tile([DK, L], F32, tag="pp", name="pp")
                        for kc in range(4):
                            nc.tensor.matmul(ps[:], wk[kc][:, hs],
                                             xk[kc][:], start=(kc == 0),
                                             stop=(kc == 3))
                        nc.vector.tensor_scalar_add(kf2[:, h, :], ps[:],
                                                    bqk[:, 8 + h:9 + h])
                    # c2p/p2c: psum halves land on partitions (j*64+r) via
                    # base-64 matmul writes, so the block-diagonal pair rhs
                    # assembles with plain same-partition copies (no shift DMA)
                    c2p2 = work.tile([128, 16, LH // 2], F16, tag="c2p2", name="c2p2")
                    p2c2 = work.tile([128, 16, L // 2], F16, tag="p2c2", name="p2c2")
                    nc.vector.memset(c2p2[:], 0.0)
                    nc.vector.memset(p2c2[:], 0.0)
                    with tc.tile_pool(name="ppc", bufs=1, space="PSUM") as ppc:
                        pc2 = ppc.tile([128, H, LH // 2], F32, tag="pc2", name="pc2")
                        pp2 = ppc.tile([128, H, L // 2], F32, tag="pp2", name="pp2")
                        for h in range(H):
                            qj = qf2[:, h, :].rearrange("d (p j) -> d j p", j=2)
                            kj = kf2[:, h, :].rearrange("d (p j) -> d j p", j=2)
                            for j in range(2):
                                nc.tensor.matmul(pc2[64 * j:64 * j + 64, h, :],
                                                 rkT[:, h, :], qj[:, j, :],
                                                 start=True, stop=True)
                                nc.tensor.matmul(pp2[64 * j:64 * j + 64, h, :],
                                                 rqT[:, h, :], kj[:, j, :],
                                                 start=True, stop=True)
                        # split the 4 assembly copies across ACT and DVE
                        nc.scalar.activation(c2p2[0:64, 0:8, :], pc2[0:64, :, :], COPY)
                        nc.scalar.activation(c2p2[64:128, 8:16, :], pc2[64:128, :, :],
                                             COPY)
                        nc.vector.tensor_copy(p2c2[0:64, 0:8, :], pp2[0:64, :, :])
                        nc.vector.tensor_copy(p2c2[64:128, 8:16, :], pp2[64:128, :, :])

                    for mc in range(2):
                        ps = pproj.tile([128, D], F32, tag="pp", name="pp")
                        for kc in range(4):
                            nc.tensor.matmul(ps[:], xv[kc][:, mc * 128:(mc + 1) * 128],
                                             wv[kc][:], start=(kc == 0),
                                             stop=(kc == 3))
                        nc.vector.tensor_add(vp[mc][:], ps[:], bvr[:])

                if phase == 1:
                    dbg_ap = qf2[:, 0, :]

            # ---------- scores + softmax ----------
            _lv = {"qk": 0, "t1": 1, "tr": 2, "B": 3, "sm": 4, "all": 9}[sub]
            if phase >= 2:
                with tc.tile_pool(name="pscore", bufs=1, space="PSUM") as pscore:
                    A = pscore.tile([128, H, L], F32, tag="A", name="A")    # 4 banks

                    # t1 first: psum t1T[mc][m, l, h], both chunks in one big
                    # tile (no WAR between chunks) -> ACT-copy to sbuf ->
                    # PE-transpose into A
                    t1s = [work.tile([128, LH, H], F32, tag=f"t1s{mc}", name=f"t1s{mc}")
                           for mc in range(2)]
                    t1 = pscore.tile([128, 2, LH, H], F32, tag="big", name="big")
                    for mc in range(2 if _lv >= 1 else 0):
                        for p in range(LH // 2):
                            nc.tensor.matmul(t1[:, mc, 2 * p:2 * p + 2, :],
                                             o1t[:, p, mc * 128:(mc + 1) * 128],
                                             c2p2[:, :, p],
                                             start=(p % 32 == 0), stop=(p % 32 == 31))
                        if mc == 0:
                            nc.vector.tensor_copy(t1s[mc][:], t1[:, mc])
                        else:
                            nc.scalar.activation(t1s[mc][:], t1[:, mc], COPY)

                    # mask seeds each bank (start=True covers 2 heads), then
                    # qk accumulates -- overlaps the ACT t1s copies
                    for h2 in range(0, H, 2):
                        nc.tensor.matmul(A[:, h2:h2 + 2, :], ones16,
                                         mask2, start=True, stop=False)
                    for h in range(H):
                        nc.tensor.matmul(A[:, h, :], qf2[:, h, :], kf2[:, h, :],
                                         start=False,
                                         stop=(_lv < 2 and h % 2 == 1))

                    # term2 before the A-transposes: B[l, m, h] one-hot
                    # matmuls, staged to SBUF in m-quarters as each accum
                    # group closes (DVE cannot read two PSUM operands on HW)
                    Bp = pscore.tile([128, L, H], F32, tag="big", name="big")
                    B_sb = work.tile([128, L, H], F16, tag="B_sb", name="B_sb")
                    for p in range(L // 2 if _lv >= 3 else 0):
                        nc.tensor.matmul(Bp[:, 2 * p:2 * p + 2, :], o2t[:, p, :],
                                         p2c2[:, :, p],
                                         start=(p % 32 == 0), stop=(p % 32 == 31))
                    if _lv >= 3:
                        nc.vector.tensor_copy(B_sb[:, 0:128, :], Bp[:, 0:128, :])
                        nc.scalar.activation(B_sb[:, 128:256, :],
                                             Bp[:, 128:256, :], COPY)

                    for mc in range(2 if _lv >= 2 else 0):
                        for h in range(H):
                            nc.tensor.matmul(A[:, h, mc * 128:(mc + 1) * 128],
                                             t1s[mc][:, :, h], id32[:],
                                             is_transpose=True, start=False,
                                             stop=(mc == 1 and h % 2 == 1))
                    # softmax per head (A from PSUM, B from SBUF).
                    # scores are bounded (~|5|) so exp needs no max-subtract;
                    # normalization cancels the missing shift exactly.
                    p16 = work.tile([128, H, L], F16, tag="p16", name="p16")
                    sums = work.tile([128, H], F32, tag="sums", name="sums")
                    recs = work.tile([128, H], F32, tag="recs", name="recs")
                    for h in range(H if _lv >= 4 else 0):
                        s = smp.tile([128, L], F16, tag="s", name="s")
                        nc.vector.tensor_add(s[:], A[:, h, :], B_sb[:, :, h])
                        e = smp.tile([128, L], F16, tag="e", name="e")
                        nc.scalar.activation(e[:], s[:], EXP, scale=1.0,
                                             accum_out=sums[:, h:h + 1])
                        nc.vector.reciprocal(recs[:, h:h + 1], sums[:, h:h + 1])
                        nc.vector.tensor_scalar_mul(p16[:, h, :], e[:], recs[:, h:h + 1])

                if phase == 2:
                    dbg_ap = {0: A[:, 0, :], 1: t1s[0][:, :, 0], 2: A[:, 0, :],
                              3: p16[:, 0, :], 4: p16[:, 0, :], 9: p16[:, 0, :]}[_lv]
                if phase == 3:
                    dbg_ap = p16[:, 0, :]

            # ---------- ctx + output projection ----------
            if phase >= 4:
                with (
                    tc.tile_pool(name="pctx", bufs=1, space="PSUM") as pctx,
                    tc.tile_pool(name="ppt", bufs=2, space="PSUM") as ppt,
                ):
                    pT = [work.tile([128, H, LH], F16, tag=f"pT{c}", name=f"pT{c}")
                          for c in range(2)]
                    for c in range(2):
                        for h in range(H):
                            pps = ppt.tile([128, 128], F16, tag="pt", name="pt")
                            nc.tensor.matmul(pps[:], p16[:, h, c * 128:(c + 1) * 128],
                                             id16[:], is_transpose=True)
                            if c == 1 and h < 6:
                                nc.scalar.activation(pT[c][:, h, :], pps[:], COPY)
                            else:
                                nc.vector.tensor_copy(pT[c][:, h, :], pps[:])

                    cp = [pctx.tile([128, 2 * LH], F32, tag=f"cp{hp}", name=f"cp{hp}")
                          for hp in range(4)]
                    nrv = LH // RVG if phase >= 5 else 0
                    for hp in range(4):
                        for c in range(2):
                            rhs = pT[c][:, 2 * hp:2 * hp + 2, :].rearrange("p hh l -> p l hh")
                            nc.tensor.matmul(cp[hp][:], vp[c][:, hp * 128:(hp + 1) * 128],
                                             rhs, start=(c == 0),
                                             stop=(c == 1 and nrv == 0))
                    for gq in range(nrv):
                        rvt = rvp.tile([128, RVG, 2, D], F8, tag="rv", name="rv")
                        eng = nc.sync if gq in RV_SP else nc.gpsimd
                        eng.dma_start(out=rvt[:], in_=d_rv[gq])
                        for g in range(RVG):
                            l = gq * RVG + g
                            for hp in range(4):
                                for c in range(2):
                                    nc.tensor.matmul(
                                        cp[hp][:, 2 * l:2 * l + 2],
                                        rvt[:, g, c, hp * 128:(hp + 1) * 128],
                                        pT[c][:, 2 * hp:2 * hp + 2, l:l + 1],
                                        start=False, stop=(c == 1 and l == LH - 1))

                    ctxT = [work.tile([128, LH], F16, tag=f"ctxT{hp}", name=f"ctxT{hp}")
                            for hp in range(4)]
                    for hp in range(4):
                        eng = nc.vector.tensor_copy if hp < 2 else (
                            lambda o, i: nc.scalar.activation(o, i, COPY))
                        eng(ctxT[hp][0:64, :],
                            cp[hp][0:64, :].rearrange("p (l hh) -> p hh l", hh=2)[:, 0, :])
                        eng(ctxT[hp][64:128, :],
                            cp[hp][64:128, :].rearrange("p (l hh) -> p hh l", hh=2)[:, 1, :])
                    # output projection in column halves so the first half's
                    # DMA overlaps the second half's matmuls
                    out_sb = work.tile([128, D], F32, tag="out_sb", name="out_sb")
                    for half in range(2):
                        cs = slice(half * 256, half * 256 + 256)
                        ops = pctx.tile([128, 256], F32, tag=f"oh{half}",
                                        name=f"oh{half}")
                        for hp in range(4):
                            nc.tensor.matmul(ops[:], ctxT[hp][:], wo[hp][:, cs],
                                             start=(hp == 0), stop=(hp == 3))
                        if half == 0:
                            nc.vector.tensor_copy(out_sb[:, cs], ops[:])
                        else:
                            nc.scalar.activation(out_sb[:, cs], ops[:], COPY)
                        nc.sync.dma_start(out=d_out[:, cs], in_=out_sb[:, cs])

            if phase < 4:
                dbg = work.tile([128, D], F32, tag="dbg", name="dbg")
                nc.vector.memset(dbg[:], 0.0)
                n = min(int(np.prod(dbg_ap.shape[1:])), D)
                nc.vector.tensor_copy(dbg[:dbg_ap.shape[0], 0:n], dbg_ap[:, 0:n])
                nc.sync.dma_start(out=d_out[:, :], in_=dbg[:])

    nc.finalize()
    return nc


_NC_CACHE = None


def _get_nc():
    global _NC_CACHE
    if _NC_CACHE is None:
        import os
        _NC_CACHE = build_nc(int(os.environ.get("KPHASE", "99")),
                             os.environ.get("KSUB", "all"))
    return _NC_CACHE


def host_prep(inputs, c):
    import ml_dtypes
    f8 = ml_dtypes.float8_e4m3
    f16, f32 = np.float16, np.float32
    b, lh = c // 2, c % 2
    l0 = lh * LH
    q = np.asarray(inputs["query"][b], f32)
    k = np.asarray(inputs["key"][b], f32)
    v = np.asarray(inputs["value"][b], f32)
    mask = np.asarray(inputs["mask"][b])
    rp = np.asarray(inputs["rel_pos"][b], np.int64)
    rv = np.asarray(inputs["rel_v"][b], f32)

    d = {}
    qT = (q[l0:l0 + LH].T * SCALE).astype(f16)          # [512, LH]
    kT = k.T.astype(f16)                                # [512, L]
    vT = v.T.astype(f16)                                # [512, L]
    d["xall"] = np.ascontiguousarray(np.concatenate(
        [qT.reshape(4, 128, LH).transpose(1, 0, 2).reshape(128, 512),
         kT.reshape(4, 128, L).transpose(1, 0, 2).reshape(128, 1024),
         vT.reshape(4, 128, L).transpose(1, 0, 2).reshape(128, 1024)], axis=1))
    WqT = np.asarray(inputs["Wq"], f32).T.astype(f16)
    WkT = np.asarray(inputs["Wk"], f32).T.astype(f16)
    WvT = np.asarray(inputs["Wv"], f32).T.astype(f16)
    WoT = np.asarray(inputs["Wo"], f32).T.astype(f16)
    d["Wall"] = np.ascontiguousarray(np.concatenate(
        [W.reshape(4, 128, D) for W in (WqT, WkT, WvT, WoT)],
        axis=0).transpose(1, 0, 2))
    mrow = np.where(mask, np.float16(MASKVAL), np.float16(0.0)).astype(f16)
    d["brow"] = np.concatenate([np.tile(mrow, 2), np.ones((256,), f16)])[None, :]
    d["bqk"] = np.ascontiguousarray(np.concatenate(
        [(np.asarray(inputs["bq"], f32) * SCALE).reshape(H, DK).T,
         np.asarray(inputs["bk"], f32).reshape(H, DK).T], axis=1)).astype(f32)
    d["bvr"] = np.tile(np.asarray(inputs["bv"], f32).astype(f16), (128, 1))
    d["rels"] = np.ascontiguousarray(np.stack(
        [np.asarray(inputs["rel_k"], f32).transpose(2, 0, 1),
         np.asarray(inputs["rel_q"], f32).transpose(2, 0, 1) * SCALE],
        axis=1)).astype(f16)
    rp_c = rp[l0:l0 + LH]
    eye = np.eye(R, dtype=f16)
    O1 = eye[:, rp_c]                                   # [R, LH, L]
    O2 = eye[:, rp_c.T]                                 # [R, L, LH]
    d["O1"] = np.ascontiguousarray(
        O1.reshape(R, LH // 2, 2, L).transpose(2, 0, 1, 3).reshape(128, LH // 2, L)
    ).astype(f8)
    d["O2"] = np.ascontiguousarray(
        O2.reshape(R, L // 2, 2, LH).transpose(2, 0, 1, 3).reshape(128, L // 2, LH)
    ).astype(f8)
    # (gq, p, g, c, f) = rv[gq*RVG+g, c*128+p, f] so each partition's group
    # payload is one contiguous run (big DMA descriptors)
    rv8 = rv[l0:l0 + LH].astype(f8)                     # [LH, L, D]
    d["rv"] = np.ascontiguousarray(
        rv8.reshape(LH // RVG, RVG, 2, 128, D).transpose(0, 3, 1, 2, 4))
    return d


def kernel(**inputs) -> np.ndarray:
    nc = _get_nc()
    in_maps = [host_prep(inputs, c) for c in range(NCORES)]
    res = run_bass_kernel_spmd(nc, in_maps, core_ids=list(range(NCORES)))
    out = np.zeros((B, L, D), np.float32)
    for c in range(NCORES):
        b, lh = c // 2, c % 2
        out[b, lh * LH:(lh + 1) * LH] = res.results[c]["out"]
    out += np.asarray(inputs["bo"], np.float32)[None, None, :]
    return out

